# revision 14
# baseline (speedup 1.0000x reference)
"""GAU attention (gated attention unit) Trainium2 Bass kernel.

Reference computation (B=2, S=2048, D=1024, H=16, DH=64):
    q = (hs @ Wq + bq), k = (hs @ Wk + bk), v = (hs @ Wv + bv)   per-head [B,S,H,DH]
    scores = q k^T / sqrt(DH);  probs = softmax(scores, axis=k)
    gating = sigmoid(gf * mean_d(hs) + gb)          # [B, S] per (batch, query)
    ctx = (probs * gating) @ v;  out = ctx @ Wo + bo

Sharding: 8 cores = 2 batches x 4 head-groups (4 heads each).  Each core
computes out^T partial [D, S] for its (batch, head-group); host sums the 4
partials per batch and adds bo.

Per-core dataflow (fp16 operands, fp32 PSUM accumulation):
  - dummy warm-up matmuls keep the PE HAM clock-gate at 2.4 GHz through the
    DMA-bound preamble, so the Q/K projections run at full rate.
  - scores^T [k,q] per (pair, ktile, qchunk): two row-packed (tile_position
    (0,0)/(64,0)) K=64 matmuls -> exp on ACT (scale=1/8) -> E^T fp16.
  - the exp chain is the pacer: at every group boundary the next group's
    first two score-pairs are hoisted ahead of the epilogue (denominators,
    scale) so ACT never stalls; Q-proj of the next chunk and O-proj of the
    previous run as sparse jobs inside the kt pipeline.
  - softmax denom: DVE folds E^T into ks_sum; two col-packed M=64 ones-
    matmuls broadcast per-head denominators into one [128,GQ] tile.
  - AV: col-packed matmuls, V stationary, both heads' ctx in ONE PSUM bank.
  - ctx^T scaled by gating*recip(denom), O-proj lhsT=Wo -> out^T staged
    fp16, DMA'd in halves.
"""

import sys

for _p in ("/opt/trn_rl_repo", "/root/.axon_site/_ro/trn_rl_repo"):
    if _p not in sys.path:
        sys.path.append(_p)

from contextlib import ExitStack

import numpy as np

import concourse.bass as bass
import concourse.mybir as mybir
import concourse.tile as tile
from concourse import bacc
from concourse.bass_utils import run_bass_kernel_spmd

F16 = mybir.dt.float16
F32 = mybir.dt.float32
AF = mybir.ActivationFunctionType
OP = mybir.AluOpType

B, S, D, H = 2, 2048, 1024, 16
DH = 64
HPC = 4  # heads per core
GD = HPC * DH  # 256 (head-group width)
NCORES = 8
NDT = D // 128  # 8 contraction tiles over D
NWARM = 40  # dummy warm-up matmuls (~9.5us at N=512 warm rate)


def _build(ctx: ExitStack, tc: "tile.TileContext", io: dict, s: int):
    nc = tc.nc
    GQ = min(512, s)
    NQC = s // GQ  # q chunks
    NKT = s // 128  # k tiles

    hsT, wq, wk, wv, wo = io["hsT"], io["wq"], io["wk"], io["wv"], io["wo"]
    bqk, bv, gr, outT = io["bqk"], io["bv"], io["gr"], io["outT"]

    consts = ctx.enter_context(tc.tile_pool(name="consts", bufs=1))
    sb = ctx.enter_context(tc.tile_pool(name="sb", bufs=1))
    etp = ctx.enter_context(tc.tile_pool(name="etp", bufs=6))
    ksp = ctx.enter_context(tc.tile_pool(name="ksp", bufs=2))
    outp = ctx.enter_context(tc.tile_pool(name="outp", bufs=2))
    # PSUM: scores 2x2 banks + ctx 2x1 (both heads share a bank) + aux 2x1 = 8
    ps_sc = ctx.enter_context(tc.tile_pool(name="ps_sc", bufs=2, space="PSUM"))
    ps_ctx = ctx.enter_context(tc.tile_pool(name="ps_ctx", bufs=2, space="PSUM"))
    ps_aux = ctx.enter_context(tc.tile_pool(name="ps_aux", bufs=2, space="PSUM"))

    # ---- constants / small inputs ----
    ones128 = consts.tile([128, 128], F16, tag="ones128", name="ones128")
    nc.vector.memset(ones128[:], 1.0)
    warm_src = consts.tile([128, GQ], F16, tag="warm", name="warm")
    nc.vector.memset(warm_src[:], 0.0)
    zbias = consts.tile([128, 1], F32, tag="zbias", name="zbias")
    nc.vector.memset(zbias[:], 0.0)

    # PE warm-up: keep the HAM clock un-throttled while DMAs land
    for w in range(NWARM):
        wp = ps_sc.tile([128, 2 * GQ], F32, tag="sc", name="warmmm")
        nc.tensor.matmul(
            wp[:, 0:GQ], lhsT=ones128[:], rhs=warm_src[:], start=True, stop=True
        )

    bqk_sb = consts.tile([128, 4], F32, tag="bqk", name="bqk")
    nc.sync.dma_start(bqk_sb[:], bqk[:, :])
    bv_bc = consts.tile([128, GD], F16, tag="bvbc", name="bvbc")
    nc.sync.dma_start(bv_bc[:], bv[:, :])
    g_sb = consts.tile([1, s], F16, tag="g", name="g")
    nc.sync.dma_start(g_sb[:], gr[:, :])

    # ---- weights + hs^T loads (order = priority: qc0's needs first) ----
    wq_sb = consts.tile([128, NDT * GD], F16, tag="wq", name="wq")
    wk_sb = consts.tile([128, NDT * GD], F16, tag="wk", name="wk")
    wv_sb = consts.tile([128, NDT * GD], F16, tag="wv", name="wv")
    wo_sb = consts.tile([128, 2 * D], F16, tag="wo", name="wo")
    hsT_sb = [sb.tile([128, s], F16, tag=f"hsT{d}", name=f"hsT{d}") for d in range(NDT)]

    nc.sync.dma_start(wq_sb[:], wq[:, :])
    nc.sync.dma_start(wk_sb[:], wk[:, :])
    half = s // 2
    for h in range(2):
        hcs = slice(h * half, (h + 1) * half)
        for d in range(NDT):
            nc.sync.dma_start(hsT_sb[d][:, hcs], hsT[d * 128 : (d + 1) * 128, hcs])
        if h == 0:
            nc.sync.dma_start(wv_sb[:], wv[:, :])
    nc.sync.dma_start(wo_sb[:], wo[:, :])

    qT_sb = [sb.tile([128, s], F16, tag=f"qT{m}", name=f"qT{m}") for m in range(2)]
    kT_sb = [sb.tile([128, s], F16, tag=f"kT{m}", name=f"kT{m}") for m in range(2)]
    v_sb = [sb.tile([128, GD], F16, tag=f"v{st}", name=f"v{st}") for st in range(NKT)]
    ctx_sc = [
        [sb.tile([128, GQ], F16, tag=f"cs{p}_{w}", name=f"cs{p}_{w}") for p in range(2)]
        for w in range(2)
    ]

    def qk_proj(w_sb, bcol, dst, m, cs):
        p = ps_aux.tile([128, GQ], F32, tag="aux", name="p")
        for d in range(NDT):
            nc.tensor.matmul(
                p[:], lhsT=w_sb[:, d * GD + m * 128 : d * GD + (m + 1) * 128],
                rhs=hsT_sb[d][:, cs], start=(d == 0), stop=(d == NDT - 1),
            )
        nc.vector.tensor_scalar_add(dst[m][:, cs], p[:], bqk_sb[:, bcol : bcol + 1])

    def v_proj(kt):
        ss = slice(kt * 128, (kt + 1) * 128)
        vp = ps_aux.tile([128, GD], F32, tag="aux", name="vp")
        for d in range(NDT):
            nc.tensor.matmul(
                vp[:], lhsT=hsT_sb[d][:, ss], rhs=wv_sb[:, d * GD : (d + 1) * GD],
                start=(d == 0), stop=(d == NDT - 1),
            )
        nc.vector.tensor_tensor(v_sb[kt][:], vp[:], bv_bc[:], op=OP.add)

    def gb_bcast(qc):
        cs = slice(qc * GQ, (qc + 1) * GQ)
        p = ps_aux.tile([128, GQ], F32, tag="aux", name="gbp")
        nc.tensor.matmul(p[:], lhsT=ones128[0:1, :], rhs=g_sb[:, cs], start=True, stop=True)
        gb = ksp.tile([128, GQ], F32, tag="gb", name="gb")
        nc.vector.tensor_copy(gb[:], p[:])
        return gb

    def emit_score_pair(qc, pr, kt):
        cs = slice(qc * GQ, (qc + 1) * GQ)
        ks_ = slice(kt * 128, (kt + 1) * 128)
        sp = ps_sc.tile([128, 2 * GQ], F32, tag="sc", name="sc")
        nc.tensor.matmul(
            sp[:, 0:GQ], lhsT=kT_sb[pr][0:64, ks_], rhs=qT_sb[pr][0:64, cs],
            tile_position=(0, 0), start=True, stop=True,
        )
        nc.tensor.matmul(
            sp[:, GQ : 2 * GQ], lhsT=kT_sb[pr][64:128, ks_], rhs=qT_sb[pr][64:128, cs],
            tile_position=(64, 0), start=True, stop=True,
        )
        return sp

    def group_compute(qc, pr, hoisted, jobs):
        """kt pipeline for one (chunk, head-pair) group; returns epilogue state.

        hoisted: pre-emitted score tiles for kt 0..len-1.  jobs: callables
        emitted sparsely inside the loop (indexed by kt).
        """
        ctx_ps = ps_ctx.tile([128, GQ], F32, tag="ctx", name="ctx")
        ks = ksp.tile([128, 2 * GQ], F16, tag="ks", name="ks")
        ets = [None] * NKT
        for kt in range(NKT + 1):
            if kt < NKT:
                if kt < len(hoisted):
                    sp = hoisted[kt]
                else:
                    sp = emit_score_pair(qc, pr, kt)
                if qc == 0 and pr == 0:
                    v_proj(kt)
            if kt > 0:
                pv = kt - 1
                et = ets[pv]
                nc.tensor.matmul(
                    ctx_ps[0:64, :], lhsT=v_sb[pv][:, pr * 128 : pr * 128 + 64],
                    rhs=et[:, 0:GQ], tile_position=(0, 0),
                    start=(pv == 0), stop=(pv == NKT - 1), skip_group_check=True,
                )
                nc.tensor.matmul(
                    ctx_ps[64:128, :], lhsT=v_sb[pv][:, pr * 128 + 64 : pr * 128 + 128],
                    rhs=et[:, GQ : 2 * GQ], tile_position=(0, 64),
                    start=(pv == 0), stop=(pv == NKT - 1), skip_group_check=True,
                )
            if kt in jobs:
                jobs[kt]()
            if kt < NKT:
                et = etp.tile([128, 2 * GQ], F16, tag="et", name="et")
                ets[kt] = et
                nc.scalar.activation(et[:], sp[:], AF.Exp, bias=zbias[:, 0:1], scale=0.125)
                if kt == 0:
                    nc.vector.tensor_copy(ks[:], et[:])
                else:
                    nc.vector.tensor_tensor(ks[:], ks[:], et[:], op=OP.add)
        return {"qc": qc, "pr": pr, "ctx_ps": ctx_ps, "ks": ks}

    def group_epilogue(st, gb):
        qc, pr, ctx_ps, ks = st["qc"], st["pr"], st["ctx_ps"], st["ks"]
        db = ps_aux.tile([128, GQ], F32, tag="aux", name="db")
        nc.tensor.matmul(
            db[0:64, :], lhsT=ones128[:, 0:64], rhs=ks[:, 0:GQ],
            tile_position=(0, 0), start=True, stop=True, skip_group_check=True,
        )
        nc.tensor.matmul(
            db[64:128, :], lhsT=ones128[:, 64:128], rhs=ks[:, GQ : 2 * GQ],
            tile_position=(0, 64), start=True, stop=True, skip_group_check=True,
        )
        r = ksp.tile([128, GQ], F32, tag="r", name="r")
        nc.vector.reciprocal_approx_fast(r[:], db[:])
        sfac = ksp.tile([128, GQ], F32, tag="sf", name="sf")
        nc.vector.tensor_tensor(sfac[:], r[:], gb[:], op=OP.mult)
        nc.vector.tensor_tensor(ctx_sc[qc % 2][pr][:], ctx_ps[:], sfac[:], op=OP.mult)

    def make_oproj_jobs(qc, tail=False):
        """O-proj of chunk qc as 4 jobs of 2 m-tiles + staged half-DMAs."""
        ot = outp.tile([128, (D // 128) * GQ], F16, tag="ot", name="ot")
        cs = slice(qc * GQ, (qc + 1) * GQ)

        def mk(mts):
            def job():
                for mt in mts:
                    o_ps = ps_aux.tile([128, GQ], F32, tag="aux", name="ops")
                    for p in range(2):
                        nc.tensor.matmul(
                            o_ps[:],
                            lhsT=wo_sb[:, p * D + mt * 128 : p * D + (mt + 1) * 128],
                            rhs=ctx_sc[qc % 2][p][:], start=(p == 0), stop=(p == 1),
                        )
                    # in the tail the exp chain is over: split evacuations
                    # between ACT and DVE so they pipeline
                    if tail and mt % 2 == 1:
                        nc.scalar.copy(ot[:, mt * GQ : (mt + 1) * GQ], o_ps[:])
                    else:
                        nc.vector.tensor_copy(ot[:, mt * GQ : (mt + 1) * GQ], o_ps[:])
                    if mt == D // 256 - 1 or mt == D // 128 - 1:
                        hn = 0 if mt == D // 256 - 1 else 1
                        nt = D // 256
                        hs_ = slice(hn * nt, (hn + 1) * nt)
                        nc.sync.dma_start(
                            outT.rearrange("(t p) s -> p t s", p=128)[:, hs_, cs],
                            ot.rearrange("p (t q) -> p t q", q=GQ)[:, hs_, :],
                        )
            return job

        nmt = D // 128
        return [mk(range(i, i + 2)) for i in range(0, nmt, 2)]

    # ---- preamble: chunk-0 Q, K chunk 0 only (rest are in-loop jobs) ----
    for m in range(2):
        qk_proj(wq_sb, m, qT_sb, m, slice(0, GQ))
    for m in range(2):
        qk_proj(wk_sb, 2 + m, kT_sb, m, slice(0, GQ))
    gb_cur = gb_bcast(0)

    def k_chunk_job(c):
        def job():
            cs = slice(c * GQ, (c + 1) * GQ)
            for m in range(2):
                qk_proj(wk_sb, 2 + m, kT_sb, m, cs)
        return job

    # ---- the chunk stream with boundary hoisting ----
    carry = []
    for qc in range(NQC):
        # pr0 jobs: qc0 projects K chunks 1..3 just ahead of their scores;
        # later chunks run the previous chunk's O-proj at kt 2,5,8,11
        jobs0 = {}
        if qc == 0:
            jobs0 = {1: k_chunk_job(1), 4: k_chunk_job(2), 8: k_chunk_job(3)}
        if qc >= 1:
            oj = make_oproj_jobs(qc - 1)
            jobs0 = {2: oj[0], 5: oj[1], 8: oj[2], 11: oj[3]}
        st0 = group_compute(qc, 0, carry, jobs0)
        hoist1 = [emit_score_pair(qc, 1, kt) for kt in range(2)]
        group_epilogue(st0, gb_cur)
        # pr1: Q-proj of the next chunk runs as jobs at kt 5,10
        jobs1 = {}
        if qc + 1 < NQC:
            ncs = slice((qc + 1) * GQ, (qc + 2) * GQ)
            jobs1 = {
                5: (lambda ncs=ncs: qk_proj(wq_sb, 0, qT_sb, 0, ncs)),
                10: (lambda ncs=ncs: qk_proj(wq_sb, 1, qT_sb, 1, ncs)),
            }
        st1 = group_compute(qc, 1, hoist1, jobs1)
        if qc + 1 < NQC:
            gb_next = gb_bcast(qc + 1)
            carry = [emit_score_pair(qc + 1, 0, kt) for kt in range(2)]
        group_epilogue(st1, gb_cur)
        if qc + 1 < NQC:
            gb_cur = gb_next

    # tail: last chunk's output projection
    for job in make_oproj_jobs(NQC - 1, tail=True):
        job()


def build_gau_nc(s: int = S, debug: bool = False):
    nc = bacc.Bacc("TRN2", target_bir_lowering=False, debug=debug, num_devices=NCORES)
    io = {
        "hsT": nc.dram_tensor("hsT", [D, s], F16, kind="ExternalInput").ap(),
        "wq": nc.dram_tensor("wq", [128, NDT * GD], F16, kind="ExternalInput").ap(),
        "wk": nc.dram_tensor("wk", [128, NDT * GD], F16, kind="ExternalInput").ap(),
        "wv": nc.dram_tensor("wv", [128, NDT * GD], F16, kind="ExternalInput").ap(),
        "wo": nc.dram_tensor("wo", [128, 2 * D], F16, kind="ExternalInput").ap(),
        "bqk": nc.dram_tensor("bqk", [128, 4], F32, kind="ExternalInput").ap(),
        "bv": nc.dram_tensor("bv", [128, GD], F16, kind="ExternalInput").ap(),
        "gr": nc.dram_tensor("gr", [1, s], F16, kind="ExternalInput").ap(),
        "outT": nc.dram_tensor("outT", [D, s], F16, kind="ExternalOutput").ap(),
    }
    with tile.TileContext(nc) as tc:
        with ExitStack() as ctx:
            _build(ctx, tc, io, s)
    nc.compile()
    return nc


def make_in_maps(hidden_states, Wq, bq, Wk, bk, Wv, bv, Wo, gating_factor, gating_bias):
    """Shard full inputs into 8 per-core input maps (host-side prep)."""
    f16 = np.float16
    f32 = np.float32
    hs = np.asarray(hidden_states, f32)
    Wq, Wk, Wv, Wo = (np.asarray(a, f32) for a in (Wq, Wk, Wv, Wo))
    bq, bk, bv = (np.asarray(a, f32) for a in (bq, bk, bv))
    gf = np.float32(np.asarray(gating_factor, f32)[0])
    gb = np.float32(np.asarray(gating_bias, f32)[0])

    hsT_b = [np.ascontiguousarray(hs[b].T).astype(f16) for b in range(B)]
    # gating row per batch: sigmoid(gf * mean_d(hs) + gb)  (host input prep)
    gx = gf * hs.mean(axis=-1) + gb  # [B, S]
    g_b = (1.0 / (1.0 + np.exp(-gx))).astype(f16)[:, None, :]  # [B, 1, S]

    def pack_w(Wcols):  # [D, GD] -> [128, NDT*GD], d-tiles side by side
        return np.ascontiguousarray(
            Wcols.reshape(NDT, 128, GD).transpose(1, 0, 2).reshape(128, NDT * GD)
        ).astype(f16)

    in_maps = []
    for c in range(NCORES):
        b, g = divmod(c, NCORES // B)
        cols = slice(g * GD, (g + 1) * GD)
        # layout per qk_proj: partition p of m-tile m holds bias element
        # m*128+p -> columns [bq_m0, bq_m1, bk_m0, bk_m1]
        bqk_pack = np.stack(
            [
                bq[cols][0:128], bq[cols][128:256],
                bk[cols][0:128], bk[cols][128:256],
            ],
            axis=1,
        ).astype(f32)
        in_maps.append(
            {
                "hsT": hsT_b[b],
                "wq": pack_w(Wq[:, cols]),
                "wk": pack_w(Wk[:, cols]),
                "wv": pack_w(Wv[:, cols]),
                "wo": np.ascontiguousarray(
                    Wo[cols, :].reshape(2, 128, D).transpose(1, 0, 2).reshape(128, 2 * D)
                ).astype(f16),
                "bqk": np.ascontiguousarray(bqk_pack),
                "bv": np.ascontiguousarray(np.broadcast_to(bv[cols], (128, GD))).astype(f16),
                "gr": np.ascontiguousarray(g_b[b]),
            }
        )
    return in_maps


_NC_CACHE: dict = {}


def _get_nc(s: int = S):
    if s not in _NC_CACHE:
        _NC_CACHE[s] = build_gau_nc(s)
    return _NC_CACHE[s]


def run_gau(in_maps, **kwargs):
    nc = _get_nc(S)
    return run_bass_kernel_spmd(nc, in_maps, core_ids=list(range(NCORES)), **kwargs)


def assemble_output(results, bo):
    """Sum per-batch head-group partials, transpose back, add bo."""
    bo = np.asarray(bo, np.float32)
    gpb = NCORES // B
    out = np.empty((B, S, D), np.float32)
    for b in range(B):
        acc = results[gpb * b]["outT"].astype(np.float32)
        for g in range(1, gpb):
            acc = acc + results[gpb * b + g]["outT"].astype(np.float32)
        out[b] = acc.T + bo[None, :]
    return out


def kernel(hidden_states, Wq, bq, Wk, bk, Wv, bv, Wo, bo, gating_factor, gating_bias):
    in_maps = make_in_maps(
        hidden_states, Wq, bq, Wk, bk, Wv, bv, Wo, gating_factor, gating_bias
    )
    res = run_gau(in_maps)
    return assemble_output(res.results, bo)


# revision 16
# speedup vs baseline: 1.0062x; 1.0062x over previous
"""GAU attention (gated attention unit) Trainium2 Bass kernel.

Reference computation (B=2, S=2048, D=1024, H=16, DH=64):
    q = (hs @ Wq + bq), k = (hs @ Wk + bk), v = (hs @ Wv + bv)   per-head [B,S,H,DH]
    scores = q k^T / sqrt(DH);  probs = softmax(scores, axis=k)
    gating = sigmoid(gf * mean_d(hs) + gb)          # [B, S] per (batch, query)
    ctx = (probs * gating) @ v;  out = ctx @ Wo + bo

Sharding: 8 cores = 2 batches x 4 head-groups (4 heads each).  Each core
computes out^T partial [D, S] for its (batch, head-group); host sums the 4
partials per batch and adds bo.

Per-core dataflow (fp16 operands, fp32 PSUM accumulation):
  - dummy warm-up matmuls keep the PE HAM clock-gate at 2.4 GHz through the
    DMA-bound preamble, so the Q/K projections run at full rate.
  - scores^T [k,q] per (pair, ktile, qchunk): two row-packed (tile_position
    (0,0)/(64,0)) K=64 matmuls -> exp on ACT (scale=1/8) -> E^T fp16.
  - the exp chain is the pacer: at every group boundary the next group's
    first two score-pairs are hoisted ahead of the epilogue (denominators,
    scale) so ACT never stalls; Q-proj of the next chunk and O-proj of the
    previous run as sparse jobs inside the kt pipeline.
  - softmax denom: DVE folds E^T into ks_sum; two col-packed M=64 ones-
    matmuls broadcast per-head denominators into one [128,GQ] tile.
  - AV: col-packed matmuls, V stationary, both heads' ctx in ONE PSUM bank.
  - ctx^T scaled by gating*recip(denom), O-proj lhsT=Wo -> out^T staged
    fp16, DMA'd in halves.
"""

import sys

for _p in ("/opt/trn_rl_repo", "/root/.axon_site/_ro/trn_rl_repo"):
    if _p not in sys.path:
        sys.path.append(_p)

from contextlib import ExitStack

import numpy as np

import concourse.bass as bass
import concourse.mybir as mybir
import concourse.tile as tile
from concourse import bacc
from concourse.bass_utils import run_bass_kernel_spmd

F16 = mybir.dt.float16
F32 = mybir.dt.float32
AF = mybir.ActivationFunctionType
OP = mybir.AluOpType

B, S, D, H = 2, 2048, 1024, 16
DH = 64
HPC = 4  # heads per core
GD = HPC * DH  # 256 (head-group width)
NCORES = 8
NDT = D // 128  # 8 contraction tiles over D
NWARM = 46  # dummy warm-up matmuls (~10us; spans the DMA-bound preamble)


def _build(ctx: ExitStack, tc: "tile.TileContext", io: dict, s: int):
    nc = tc.nc
    GQ = min(512, s)
    NQC = s // GQ  # q chunks
    NKT = s // 128  # k tiles

    hsT, wq, wk, wv, wo = io["hsT"], io["wq"], io["wk"], io["wv"], io["wo"]
    bqk, bv, gr, outT = io["bqk"], io["bv"], io["gr"], io["outT"]

    consts = ctx.enter_context(tc.tile_pool(name="consts", bufs=1))
    sb = ctx.enter_context(tc.tile_pool(name="sb", bufs=1))
    etp = ctx.enter_context(tc.tile_pool(name="etp", bufs=6))
    ksp = ctx.enter_context(tc.tile_pool(name="ksp", bufs=2))
    outp = ctx.enter_context(tc.tile_pool(name="outp", bufs=2))
    # PSUM: scores 2x2 banks + ctx 2x1 (both heads share a bank) + aux 2x1 = 8
    ps_sc = ctx.enter_context(tc.tile_pool(name="ps_sc", bufs=2, space="PSUM"))
    ps_ctx = ctx.enter_context(tc.tile_pool(name="ps_ctx", bufs=2, space="PSUM"))
    ps_aux = ctx.enter_context(tc.tile_pool(name="ps_aux", bufs=2, space="PSUM"))

    # ---- constants / small inputs ----
    ones128 = consts.tile([128, 128], F16, tag="ones128", name="ones128")
    nc.vector.memset(ones128[:], 1.0)
    warm_src = consts.tile([128, GQ], F16, tag="warm", name="warm")
    nc.vector.memset(warm_src[:], 0.0)
    zbias = consts.tile([128, 1], F32, tag="zbias", name="zbias")
    nc.vector.memset(zbias[:], 0.0)

    # PE warm-up: keep the HAM clock un-throttled while DMAs land
    for w in range(NWARM):
        wp = ps_sc.tile([128, 2 * GQ], F32, tag="sc", name="warmmm")
        nc.tensor.matmul(
            wp[:, 0:GQ], lhsT=ones128[:], rhs=warm_src[:], start=True, stop=True
        )

    bqk_sb = consts.tile([128, 4], F32, tag="bqk", name="bqk")
    nc.sync.dma_start(bqk_sb[:], bqk[:, :])
    bv_bc = consts.tile([128, GD], F16, tag="bvbc", name="bvbc")
    nc.sync.dma_start(bv_bc[:], bv[:, :])
    g_sb = consts.tile([1, s], F16, tag="g", name="g")
    nc.sync.dma_start(g_sb[:], gr[:, :])

    # ---- weights + hs^T loads (order = priority: qc0's needs first) ----
    wq_sb = consts.tile([128, NDT * GD], F16, tag="wq", name="wq")
    wk_sb = consts.tile([128, NDT * GD], F16, tag="wk", name="wk")
    wv_sb = consts.tile([128, NDT * GD], F16, tag="wv", name="wv")
    wo_sb = consts.tile([128, 2 * D], F16, tag="wo", name="wo")
    hsT_sb = [sb.tile([128, s], F16, tag=f"hsT{d}", name=f"hsT{d}") for d in range(NDT)]

    nc.sync.dma_start(wq_sb[:], wq[:, :])
    nc.sync.dma_start(wk_sb[:], wk[:, :])
    half = s // 2
    for h in range(2):
        hcs = slice(h * half, (h + 1) * half)
        for d in range(NDT):
            nc.sync.dma_start(hsT_sb[d][:, hcs], hsT[d * 128 : (d + 1) * 128, hcs])
        if h == 0:
            nc.sync.dma_start(wv_sb[:], wv[:, :])
    nc.sync.dma_start(wo_sb[:], wo[:, :])

    qT_sb = [sb.tile([128, s], F16, tag=f"qT{m}", name=f"qT{m}") for m in range(2)]
    kT_sb = [sb.tile([128, s], F16, tag=f"kT{m}", name=f"kT{m}") for m in range(2)]
    v_sb = [sb.tile([128, GD], F16, tag=f"v{st}", name=f"v{st}") for st in range(NKT)]
    ctx_sc = [
        [sb.tile([128, GQ], F16, tag=f"cs{p}_{w}", name=f"cs{p}_{w}") for p in range(2)]
        for w in range(2)
    ]

    def qk_proj(w_sb, bcol, dst, m, cs):
        p = ps_aux.tile([128, GQ], F32, tag="aux", name="p")
        for d in range(NDT):
            nc.tensor.matmul(
                p[:], lhsT=w_sb[:, d * GD + m * 128 : d * GD + (m + 1) * 128],
                rhs=hsT_sb[d][:, cs], start=(d == 0), stop=(d == NDT - 1),
            )
        nc.vector.tensor_scalar_add(dst[m][:, cs], p[:], bqk_sb[:, bcol : bcol + 1])

    def v_proj(kt):
        ss = slice(kt * 128, (kt + 1) * 128)
        vp = ps_aux.tile([128, GD], F32, tag="aux", name="vp")
        for d in range(NDT):
            nc.tensor.matmul(
                vp[:], lhsT=hsT_sb[d][:, ss], rhs=wv_sb[:, d * GD : (d + 1) * GD],
                start=(d == 0), stop=(d == NDT - 1),
            )
        nc.vector.tensor_tensor(v_sb[kt][:], vp[:], bv_bc[:], op=OP.add)

    def gb_bcast(qc):
        cs = slice(qc * GQ, (qc + 1) * GQ)
        p = ps_aux.tile([128, GQ], F32, tag="aux", name="gbp")
        nc.tensor.matmul(p[:], lhsT=ones128[0:1, :], rhs=g_sb[:, cs], start=True, stop=True)
        gb = ksp.tile([128, GQ], F32, tag="gb", name="gb")
        nc.vector.tensor_copy(gb[:], p[:])
        return gb

    def emit_score_pair(qc, pr, kt):
        cs = slice(qc * GQ, (qc + 1) * GQ)
        ks_ = slice(kt * 128, (kt + 1) * 128)
        sp = ps_sc.tile([128, 2 * GQ], F32, tag="sc", name="sc")
        nc.tensor.matmul(
            sp[:, 0:GQ], lhsT=kT_sb[pr][0:64, ks_], rhs=qT_sb[pr][0:64, cs],
            tile_position=(0, 0), start=True, stop=True,
        )
        nc.tensor.matmul(
            sp[:, GQ : 2 * GQ], lhsT=kT_sb[pr][64:128, ks_], rhs=qT_sb[pr][64:128, cs],
            tile_position=(64, 0), start=True, stop=True,
        )
        return sp

    def group_compute(qc, pr, hoisted, jobs):
        """kt pipeline for one (chunk, head-pair) group; returns epilogue state.

        hoisted: pre-emitted score tiles for kt 0..len-1.  jobs: callables
        emitted sparsely inside the loop (indexed by kt).
        """
        ctx_ps = ps_ctx.tile([128, GQ], F32, tag="ctx", name="ctx")
        ks = ksp.tile([128, 2 * GQ], F16, tag="ks", name="ks")
        ets = [None] * NKT
        for kt in range(NKT + 1):
            if kt < NKT:
                if kt < len(hoisted):
                    sp = hoisted[kt]
                else:
                    sp = emit_score_pair(qc, pr, kt)
                if qc == 0 and pr == 0:
                    v_proj(kt)
            if kt > 0:
                pv = kt - 1
                et = ets[pv]
                nc.tensor.matmul(
                    ctx_ps[0:64, :], lhsT=v_sb[pv][:, pr * 128 : pr * 128 + 64],
                    rhs=et[:, 0:GQ], tile_position=(0, 0),
                    start=(pv == 0), stop=(pv == NKT - 1), skip_group_check=True,
                )
                nc.tensor.matmul(
                    ctx_ps[64:128, :], lhsT=v_sb[pv][:, pr * 128 + 64 : pr * 128 + 128],
                    rhs=et[:, GQ : 2 * GQ], tile_position=(0, 64),
                    start=(pv == 0), stop=(pv == NKT - 1), skip_group_check=True,
                )
            if kt in jobs:
                jobs[kt]()
            if kt < NKT:
                et = etp.tile([128, 2 * GQ], F16, tag="et", name="et")
                ets[kt] = et
                nc.scalar.activation(et[:], sp[:], AF.Exp, bias=zbias[:, 0:1], scale=0.125)
                if kt == 0:
                    nc.vector.tensor_copy(ks[:], et[:])
                else:
                    nc.vector.tensor_tensor(ks[:], ks[:], et[:], op=OP.add)
        return {"qc": qc, "pr": pr, "ctx_ps": ctx_ps, "ks": ks}

    def group_epilogue(st, gb):
        qc, pr, ctx_ps, ks = st["qc"], st["pr"], st["ctx_ps"], st["ks"]
        db = ps_aux.tile([128, GQ], F32, tag="aux", name="db")
        nc.tensor.matmul(
            db[0:64, :], lhsT=ones128[:, 0:64], rhs=ks[:, 0:GQ],
            tile_position=(0, 0), start=True, stop=True, skip_group_check=True,
        )
        nc.tensor.matmul(
            db[64:128, :], lhsT=ones128[:, 64:128], rhs=ks[:, GQ : 2 * GQ],
            tile_position=(0, 64), start=True, stop=True, skip_group_check=True,
        )
        r = ksp.tile([128, GQ], F32, tag="r", name="r")
        nc.vector.reciprocal_approx_fast(r[:], db[:])
        sfac = ksp.tile([128, GQ], F32, tag="sf", name="sf")
        nc.vector.tensor_tensor(sfac[:], r[:], gb[:], op=OP.mult)
        nc.vector.tensor_tensor(ctx_sc[qc % 2][pr][:], ctx_ps[:], sfac[:], op=OP.mult)

    def make_oproj_jobs(qc, tail=False):
        """O-proj of chunk qc as 4 jobs of 2 m-tiles + staged half-DMAs."""
        ot = outp.tile([128, (D // 128) * GQ], F16, tag="ot", name="ot")
        cs = slice(qc * GQ, (qc + 1) * GQ)

        def mk(mts):
            def job():
                for mt in mts:
                    o_ps = ps_aux.tile([128, GQ], F32, tag="aux", name="ops")
                    for p in range(2):
                        nc.tensor.matmul(
                            o_ps[:],
                            lhsT=wo_sb[:, p * D + mt * 128 : p * D + (mt + 1) * 128],
                            rhs=ctx_sc[qc % 2][p][:], start=(p == 0), stop=(p == 1),
                        )
                    # in the tail the exp chain is over: split evacuations
                    # between ACT and DVE so they pipeline
                    if tail and mt % 2 == 1:
                        nc.scalar.copy(ot[:, mt * GQ : (mt + 1) * GQ], o_ps[:])
                    else:
                        nc.vector.tensor_copy(ot[:, mt * GQ : (mt + 1) * GQ], o_ps[:])
                    if mt == D // 256 - 1 or mt == D // 128 - 1:
                        hn = 0 if mt == D // 256 - 1 else 1
                        nt = D // 256
                        hs_ = slice(hn * nt, (hn + 1) * nt)
                        nc.sync.dma_start(
                            outT.rearrange("(t p) s -> p t s", p=128)[:, hs_, cs],
                            ot.rearrange("p (t q) -> p t q", q=GQ)[:, hs_, :],
                        )
            return job

        nmt = D // 128
        return [mk(range(i, i + 2)) for i in range(0, nmt, 2)]

    # ---- preamble: chunk-0 Q, all K ----
    for m in range(2):
        qk_proj(wq_sb, m, qT_sb, m, slice(0, GQ))
    for qc in range(NQC):
        cs = slice(qc * GQ, (qc + 1) * GQ)
        for m in range(2):
            qk_proj(wk_sb, 2 + m, kT_sb, m, cs)
    gb_cur = gb_bcast(0)

    # ---- the chunk stream with boundary hoisting ----
    carry = []
    for qc in range(NQC):
        # pr0: O-proj of the previous chunk runs as jobs at kt 2,5,8,11
        jobs0 = {}
        if qc >= 1:
            oj = make_oproj_jobs(qc - 1)
            jobs0 = {2: oj[0], 5: oj[1], 8: oj[2], 11: oj[3]}
        st0 = group_compute(qc, 0, carry, jobs0)
        hoist1 = [emit_score_pair(qc, 1, kt) for kt in range(2)]
        group_epilogue(st0, gb_cur)
        # pr1: Q-proj of the next chunk runs as jobs at kt 5,10
        jobs1 = {}
        if qc + 1 < NQC:
            ncs = slice((qc + 1) * GQ, (qc + 2) * GQ)
            jobs1 = {
                5: (lambda ncs=ncs: qk_proj(wq_sb, 0, qT_sb, 0, ncs)),
                10: (lambda ncs=ncs: qk_proj(wq_sb, 1, qT_sb, 1, ncs)),
            }
        st1 = group_compute(qc, 1, hoist1, jobs1)
        if qc + 1 < NQC:
            gb_next = gb_bcast(qc + 1)
            carry = [emit_score_pair(qc + 1, 0, kt) for kt in range(2)]
        group_epilogue(st1, gb_cur)
        if qc + 1 < NQC:
            gb_cur = gb_next

    # tail: last chunk's output projection
    for job in make_oproj_jobs(NQC - 1, tail=True):
        job()


def build_gau_nc(s: int = S, debug: bool = False):
    nc = bacc.Bacc("TRN2", target_bir_lowering=False, debug=debug, num_devices=NCORES)
    io = {
        "hsT": nc.dram_tensor("hsT", [D, s], F16, kind="ExternalInput").ap(),
        "wq": nc.dram_tensor("wq", [128, NDT * GD], F16, kind="ExternalInput").ap(),
        "wk": nc.dram_tensor("wk", [128, NDT * GD], F16, kind="ExternalInput").ap(),
        "wv": nc.dram_tensor("wv", [128, NDT * GD], F16, kind="ExternalInput").ap(),
        "wo": nc.dram_tensor("wo", [128, 2 * D], F16, kind="ExternalInput").ap(),
        "bqk": nc.dram_tensor("bqk", [128, 4], F32, kind="ExternalInput").ap(),
        "bv": nc.dram_tensor("bv", [128, GD], F16, kind="ExternalInput").ap(),
        "gr": nc.dram_tensor("gr", [1, s], F16, kind="ExternalInput").ap(),
        "outT": nc.dram_tensor("outT", [D, s], F16, kind="ExternalOutput").ap(),
    }
    with tile.TileContext(nc) as tc:
        with ExitStack() as ctx:
            _build(ctx, tc, io, s)
    nc.compile()
    return nc


def make_in_maps(hidden_states, Wq, bq, Wk, bk, Wv, bv, Wo, gating_factor, gating_bias):
    """Shard full inputs into 8 per-core input maps (host-side prep)."""
    f16 = np.float16
    f32 = np.float32
    hs = np.asarray(hidden_states, f32)
    Wq, Wk, Wv, Wo = (np.asarray(a, f32) for a in (Wq, Wk, Wv, Wo))
    bq, bk, bv = (np.asarray(a, f32) for a in (bq, bk, bv))
    gf = np.float32(np.asarray(gating_factor, f32)[0])
    gb = np.float32(np.asarray(gating_bias, f32)[0])

    hsT_b = [np.ascontiguousarray(hs[b].T).astype(f16) for b in range(B)]
    # gating row per batch: sigmoid(gf * mean_d(hs) + gb)  (host input prep)
    gx = gf * hs.mean(axis=-1) + gb  # [B, S]
    g_b = (1.0 / (1.0 + np.exp(-gx))).astype(f16)[:, None, :]  # [B, 1, S]

    def pack_w(Wcols):  # [D, GD] -> [128, NDT*GD], d-tiles side by side
        return np.ascontiguousarray(
            Wcols.reshape(NDT, 128, GD).transpose(1, 0, 2).reshape(128, NDT * GD)
        ).astype(f16)

    in_maps = []
    for c in range(NCORES):
        b, g = divmod(c, NCORES // B)
        cols = slice(g * GD, (g + 1) * GD)
        # layout per qk_proj: partition p of m-tile m holds bias element
        # m*128+p -> columns [bq_m0, bq_m1, bk_m0, bk_m1]
        bqk_pack = np.stack(
            [
                bq[cols][0:128], bq[cols][128:256],
                bk[cols][0:128], bk[cols][128:256],
            ],
            axis=1,
        ).astype(f32)
        in_maps.append(
            {
                "hsT": hsT_b[b],
                "wq": pack_w(Wq[:, cols]),
                "wk": pack_w(Wk[:, cols]),
                "wv": pack_w(Wv[:, cols]),
                "wo": np.ascontiguousarray(
                    Wo[cols, :].reshape(2, 128, D).transpose(1, 0, 2).reshape(128, 2 * D)
                ).astype(f16),
                "bqk": np.ascontiguousarray(bqk_pack),
                "bv": np.ascontiguousarray(np.broadcast_to(bv[cols], (128, GD))).astype(f16),
                "gr": np.ascontiguousarray(g_b[b]),
            }
        )
    return in_maps


_NC_CACHE: dict = {}


def _get_nc(s: int = S):
    if s not in _NC_CACHE:
        _NC_CACHE[s] = build_gau_nc(s)
    return _NC_CACHE[s]


def run_gau(in_maps, **kwargs):
    nc = _get_nc(S)
    return run_bass_kernel_spmd(nc, in_maps, core_ids=list(range(NCORES)), **kwargs)


def assemble_output(results, bo):
    """Sum per-batch head-group partials, transpose back, add bo."""
    bo = np.asarray(bo, np.float32)
    gpb = NCORES // B
    out = np.empty((B, S, D), np.float32)
    for b in range(B):
        acc = results[gpb * b]["outT"].astype(np.float32)
        for g in range(1, gpb):
            acc = acc + results[gpb * b + g]["outT"].astype(np.float32)
        out[b] = acc.T + bo[None, :]
    return out


def kernel(hidden_states, Wq, bq, Wk, bk, Wv, bv, Wo, bo, gating_factor, gating_bias):
    in_maps = make_in_maps(
        hidden_states, Wq, bq, Wk, bk, Wv, bv, Wo, gating_factor, gating_bias
    )
    res = run_gau(in_maps)
    return assemble_output(res.results, bo)
